# revision 25
# baseline (speedup 1.0000x reference)
"""Trainium2 Bass kernel for an RNN-T style joint network MLP.

  out[b,t,u,o] = tanh(enc[b,t,:] @ W1[:512] + dec[b,u,:] @ W1[512:] + b1) @ W2 + b2

Shapes: enc (8, 256, 512), dec (8, 64, 512), W1 (1024, 1024), b1 (1024,),
W2 (1024, 128), b2 (128,), out (8, 256, 64, 128), all float32.

Sharding: data-parallel over batch — one batch element per NeuronCore,
no collectives. Per core:
  - PE: e_projT[h,t], dec_projT[h,u] (bf16 in, fp32 accum), main GEMM
        psum[o, (u0 t|u1 t)] = sum_h W2[h,o] * tanh[h, ...] in bf16, N=512.
  - DVE: sum[h,t] = e_projT[h,t] + (dec_projT[h,u]+b1[h])  (tensor_scalar_add
        bf16 4x mode, per-partition scalar), and PSUM evacuation with +b2
        fused (b2 is per-partition in the [o,t] layout).
  - ACT: one big tanh per u-block ([128, UB*2048]) to amortize per-op cost.
Output layout on device is outT[o,u,t] per core; host transposes on gather.

Measured (8 axon trn2 cores): ~155 us HW exec, rel err ~3.6e-3 vs fp32
reference (bf16 GEMM datapath; PSUM accumulation in fp32). Steady state is
co-limited by ACT tanh (~112 us: 16.8M tanh/core at 1 elem/cycle/lane
@1.2GHz) and DVE bias-adds+evac (~119 us busy).
"""

import os
import numpy as np
import ml_dtypes

B, T, U, D, H, O = 8, 256, 64, 512, 1024, 128
NCORES = 8
UB = 4            # u-block size (pipeline granularity)
HC = H // 128     # 8 h-chunks
DC = 2 * D // 128 # 8 d-chunks of W1 (4 enc + 4 dec)

_CACHE = {}
LAST_RESULT = None  # BassKernelResults from the most recent run (for profiling)


def _build_program():
    from concourse import bacc, tile
    import concourse.mybir as mybir

    dt = mybir.dt
    f32, bf16 = dt.float32, dt.bfloat16
    Act = mybir.ActivationFunctionType

    nc = bacc.Bacc("TRN2", target_bir_lowering=False, debug=False)

    encT = nc.dram_tensor("encT", [D, T], bf16, kind="ExternalInput").ap()
    decT = nc.dram_tensor("decT", [D, U], bf16, kind="ExternalInput").ap()
    W1 = nc.dram_tensor("W1", [2 * D, H], bf16, kind="ExternalInput").ap()
    W2bf = nc.dram_tensor("W2bf", [H, O], bf16, kind="ExternalInput").ap()
    b1r = nc.dram_tensor("b1r", [128, HC], f32, kind="ExternalInput").ap()
    b2c = nc.dram_tensor("b2c", [O, 1], f32, kind="ExternalInput").ap()
    outT = nc.dram_tensor("outT", [O, U, T], f32, kind="ExternalOutput").ap()

    BW = UB * 2048  # per-block sum/tanh width (hc-major: [hc][u][t])

    with tile.TileContext(nc) as tc:
        with tc.tile_pool(name="persist", bufs=1) as persist, \
             tc.tile_pool(name="sums", bufs=4) as sums_pool, \
             tc.tile_pool(name="tanhp", bufs=3) as tanh_pool, \
             tc.tile_pool(name="outsb", bufs=4) as out_pool, \
             tc.tile_pool(name="psum", bufs=8, space="PSUM") as psum_pool:

            w1_sb = persist.tile([128, DC * H], bf16, tag="w1")
            encT_sb = persist.tile([128, 4 * T], bf16, tag="encT")
            decT_sb = persist.tile([128, 4 * U], bf16, tag="decT")
            w2_sb = persist.tile([128, HC * O], bf16, tag="w2")
            b1_sb = persist.tile([128, HC], f32, tag="b1")
            b2_sb = persist.tile([128, 1], f32, tag="b2")
            e_sb = persist.tile([128, HC * T], bf16, tag="eproj")
            bias_sb = persist.tile([128, HC * U], f32, tag="bias")

            # ---- loads: coalesced (few big DMAs; ~650ns queue cost each).
            # enc-path tensors first: the enc GEMM is the head's long pole
            # (32 cold matmuls), so its weights must land earliest.
            nc.sync.dma_start(
                encT_sb[:, :].rearrange("p (c t) -> p c t", c=4),
                encT[:, :].rearrange("(c p) t -> p c t", p=128))
            nc.sync.dma_start(
                w1_sb[:, 0:4 * H].rearrange("p (c h) -> p c h", c=4),
                W1[0:512, :].rearrange("(c p) h -> p c h", p=128))
            nc.sync.dma_start(
                w1_sb[:, 4 * H:8 * H].rearrange("p (c h) -> p c h", c=4),
                W1[512:1024, :].rearrange("(c p) h -> p c h", p=128))
            nc.sync.dma_start(
                decT_sb[:, :].rearrange("p (c u) -> p c u", c=4),
                decT[:, :].rearrange("(c p) u -> p c u", p=128))
            nc.sync.dma_start(b1_sb[:], b1r[:, :])
            nc.sync.dma_start(b2_sb[:], b2c[:, :])
            nc.sync.dma_start(
                w2_sb[:, :].rearrange("p (c o) -> p c o", c=HC),
                W2bf[:, :].rearrange("(c p) o -> p c o", p=128))

            # ---- first GEMMs, interleaved per h-chunk so PSUM slots recycle
            # enc: e_projT[h,t] = sum_d W_enc[d,h]*encT[d,t]
            # dec: bias[h,u] = sum_d W_dec[d,h]*decT[d,u] + b1[h]
            for hc in range(HC):
                pe = psum_pool.tile([128, T], f32, tag="ps", name=f"pe{hc}")
                for dc in range(4):
                    nc.tensor.matmul(
                        pe[:],
                        lhsT=w1_sb[:, dc * H + hc * 128: dc * H + hc * 128 + 128],
                        rhs=encT_sb[:, dc * T:(dc + 1) * T],
                        start=(dc == 0), stop=(dc == 3),
                    )
                nc.vector.tensor_copy(e_sb[:, hc * T:(hc + 1) * T], pe[:])

                pd = psum_pool.tile([128, U], f32, tag="ps", name=f"pd{hc}")
                for dc in range(4):
                    nc.tensor.matmul(
                        pd[:],
                        lhsT=w1_sb[:, (4 + dc) * H + hc * 128: (4 + dc) * H + hc * 128 + 128],
                        rhs=decT_sb[:, dc * U:(dc + 1) * U],
                        start=(dc == 0), stop=(dc == 3),
                    )
                nc.scalar.activation(bias_sb[:, hc * U:(hc + 1) * U], pd[:],
                                     Act.Identity, bias=b1_sb[:, hc:hc + 1])

            # ---- main pipeline over u-blocks ----
            # sum/tanh layout per block: [hc][u][t] (hc-major) so a u-pair is
            # contiguous and the main GEMM can run N=512 into one PSUM bank.
            NB = U // UB
            NP = UB // 2  # u-pairs per block
            for blk in range(NB):
                sum_sb = sums_pool.tile([128, BW], bf16, tag="sum")
                for hc in range(HC):
                    for ul in range(UB):
                        u = blk * UB + ul
                        nc.vector.tensor_scalar_add(
                            sum_sb[:, hc * (UB * T) + ul * T: hc * (UB * T) + ul * T + T],
                            e_sb[:, hc * T:(hc + 1) * T],
                            bias_sb[:, hc * U + u: hc * U + u + 1],
                        )

                tanh_sb = tanh_pool.tile([128, BW], bf16, tag="tanh")
                if blk in (0, NB - 1):
                    # split tanh in halves at the pipeline head/tail so the
                    # PE can start (resp. finish) half a block earlier
                    nc.scalar.activation(tanh_sb[:, :BW // 2],
                                         sum_sb[:, :BW // 2], Act.Tanh)
                    nc.scalar.activation(tanh_sb[:, BW // 2:],
                                         sum_sb[:, BW // 2:], Act.Tanh)
                else:
                    nc.scalar.activation(tanh_sb[:], sum_sb[:], Act.Tanh)

                pos = [psum_pool.tile([128, 2 * T], f32, tag="ps", name=f"po{blk}_{p}")
                       for p in range(NP)]
                for hc in range(HC):  # hc outer: W2 chunk stays stationary
                    for p in range(NP):
                        nc.tensor.matmul(
                            pos[p][:],
                            lhsT=w2_sb[:, hc * O:(hc + 1) * O],
                            rhs=tanh_sb[:, hc * (UB * T) + p * 2 * T: hc * (UB * T) + (p + 1) * 2 * T],
                            start=(hc == 0), stop=(hc == HC - 1),
                        )

                out_sb = out_pool.tile([128, UB * T], f32, tag="osb")
                for p in range(NP):
                    # DVE paces the steady state (7.59us/blk vs ACT 7.01):
                    # shift one pair-evac per two blocks onto ACT to balance.
                    if p == 0 and (blk % 2 == 0 or blk == NB - 1):
                        nc.scalar.activation(
                            out_sb[:, p * 2 * T:(p + 1) * 2 * T], pos[p][:],
                            Act.Identity, bias=b2_sb[:, 0:1])
                    else:
                        nc.vector.tensor_scalar_add(
                            out_sb[:, p * 2 * T:(p + 1) * 2 * T], pos[p][:],
                            b2_sb[:, 0:1])

                if blk == NB - 1:
                    # split the final store so the first pair's DMA starts
                    # as soon as its evac lands
                    nc.sync.dma_start(outT[:, blk * UB:blk * UB + 2, :],
                                      out_sb[:, 0:2 * T])
                    nc.sync.dma_start(outT[:, blk * UB + 2:(blk + 1) * UB, :],
                                      out_sb[:, 2 * T:])
                else:
                    nc.sync.dma_start(outT[:, blk * UB:(blk + 1) * UB, :],
                                      out_sb[:])

    nc.compile()
    return nc


def kernel(encoder_state, decoder_state, W1, b1, W2, b2):
    from concourse.bass_utils import run_bass_kernel_spmd
    global LAST_RESULT

    if "nc" not in _CACHE:
        _CACHE["nc"] = _build_program()
    nc = _CACHE["nc"]

    encoder_state = np.asarray(encoder_state, dtype=np.float32)
    decoder_state = np.asarray(decoder_state, dtype=np.float32)
    W1 = np.asarray(W1, dtype=np.float32)
    b1 = np.asarray(b1, dtype=np.float32)
    W2 = np.asarray(W2, dtype=np.float32)
    b2 = np.asarray(b2, dtype=np.float32)

    bf = ml_dtypes.bfloat16
    W1bf = W1.astype(bf)
    W2bf = W2.astype(bf)
    b1r = np.ascontiguousarray(b1.reshape(HC, 128).T)  # [128, 8]
    b2c = np.ascontiguousarray(b2.reshape(O, 1))

    in_maps = []
    for i in range(NCORES):
        in_maps.append({
            "encT": np.ascontiguousarray(encoder_state[i].T.astype(bf)),  # [512, 256]
            "decT": np.ascontiguousarray(decoder_state[i].T.astype(bf)),  # [512, 64]
            "W1": W1bf,
            "W2bf": W2bf,
            "b1r": b1r,
            "b2c": b2c,
        })

    trace = bool(int(os.environ.get("KERNEL_TRACE", "0")))
    res = run_bass_kernel_spmd(nc, in_maps, list(range(NCORES)), trace=trace)
    LAST_RESULT = res

    # gather: outT[core] is [O, U, T] -> out[b, t, u, o]
    out = np.empty((B, T, U, O), dtype=np.float32)
    for i in range(NCORES):
        out[i] = res.results[i]["outT"].transpose(2, 1, 0)
    return out
